# revision 1
# baseline (speedup 1.0000x reference)
"""GCN encoder kernel for trn2 (8 cores).

Math (reference):
    src,dst,norm = gcn_norm(edge_index, N)   # self loops + sym norm
    h  = x @ W_ae + b_ae
    h1 = sigmoid(Ahat @ (h @ W1) + b1)
    y  = sigmoid(Ahat @ (h1 @ W2) + b2)

Restructured (identical math):
    W_c = W_ae @ W1, b_c = b_ae @ W1, r = Ahat @ 1
    g1 = Ahat @ x                       # sparse, 256-dim gather
    h1 = sigmoid(g1 @ W_c + r b_c^T + 1 b1^T)
    hw2 = h1 @ W2
    y  = sigmoid(Ahat @ hw2 + 1 b2^T)   # sparse, 256-dim gather

Sharding: core c owns dst rows [c*R, (c+1)*R).  One AllGather of hw2.
Sparse phase: dma_gather rows from HBM, segment-sum via PE matmul with
per-tile selection matrices S (norm folded in) built on DVE.
"""

import dataclasses
import math
from contextlib import ExitStack

import numpy as np

import concourse.bass as bass
import concourse.mybir as mybir
import concourse.tile as tile
from concourse import bacc

F16 = mybir.dt.float16
F32 = mybir.dt.float32
I16 = mybir.dt.int16


@dataclasses.dataclass
class Cfg:
    N: int = 50000
    E: int = 1600000
    IN: int = 256          # gather feature dim (x and hw2)
    HID: int = 512         # h1 dim
    OUT: int = 256         # output dim
    n_cores: int = 8
    HALF: int = 32768      # int16 index split point
    WROWS: int = 128       # dst rows per window
    CHUNK: int = 512       # dense matmul node-chunk (PSUM free dim)

    @property
    def R(self):
        assert self.N % self.n_cores == 0
        return self.N // self.n_cores

    @property
    def n_win(self):
        return (self.R + self.WROWS - 1) // self.WROWS


def preprocess(cfg: Cfg, x, edge_index, W_ae, b_ae, W1, b1, W2, b2):
    """Host-side: graph norm, weight folding, per-core edge layout."""
    N, R = cfg.N, cfg.R
    src = np.asarray(edge_index[0], dtype=np.int64)
    dst = np.asarray(edge_index[1], dtype=np.int64)
    loops = np.arange(N, dtype=np.int64)
    src = np.concatenate([src, loops])
    dst = np.concatenate([dst, loops])
    deg = np.bincount(dst, minlength=N).astype(np.float32)
    dinv = np.where(deg > 0, 1.0 / np.sqrt(deg), 0.0).astype(np.float32)
    norm = (dinv[src] * dinv[dst]).astype(np.float32)
    r_vec = np.bincount(dst, weights=norm.astype(np.float64), minlength=N)
    r_vec = r_vec.astype(np.float32)

    W_c = (np.asarray(W_ae, np.float32) @ np.asarray(W1, np.float32))
    b_c = (np.asarray(b_ae, np.float32) @ np.asarray(W1, np.float32))
    b1 = np.asarray(b1, np.float32)
    b2 = np.asarray(b2, np.float32)

    # per (core, window, group) edge lists
    core_of = dst // R
    wrow = (dst % R) // cfg.WROWS
    grp = (src >= cfg.HALF).astype(np.int64)
    n_win = cfg.n_win
    # count per (core, window, group)
    key = (core_of * n_win + wrow) * 2 + grp
    counts = np.bincount(key, minlength=cfg.n_cores * n_win * 2).reshape(
        cfg.n_cores, n_win, 2
    )
    # per-window tile counts, max over cores (uniform SPMD program)
    TAs = np.ceil(counts[:, :, 0].max(axis=0) / 128).astype(np.int64)  # [n_win]
    TBs = np.ceil(counts[:, :, 1].max(axis=0) / 128).astype(np.int64)
    TWs = TAs + TBs
    tile_base = np.zeros(n_win + 1, np.int64)
    tile_base[1:] = np.cumsum(TWs)
    T_total = int(tile_base[-1])
    S_total = T_total * 128
    # pad S_total to a multiple of 16 cols is automatic (128 | S_total)
    TWmax = int(TWs.max())

    # order edges by key then src (sorted src within group => quasi-seq HBM reads)
    order = np.lexsort((src, key))
    s_src, s_dst, s_norm, s_key = src[order], dst[order], norm[order], key[order]
    # slot id within each key block
    blk_start = np.searchsorted(s_key, np.arange(cfg.n_cores * n_win * 2))
    within = np.arange(len(s_src)) - blk_start[s_key]

    idx_all = np.full((cfg.n_cores, S_total), -1, np.int16)
    dst_all = np.zeros((cfg.n_cores, S_total), np.float32)
    nrm_all = np.zeros((cfg.n_cores, S_total), np.float32)

    c_arr = s_key // (2 * n_win)
    w_arr = (s_key // 2) % n_win
    g_arr = s_key % 2
    slot = (tile_base[w_arr] + g_arr * TAs[w_arr]) * 128 + within
    idx16 = np.where(g_arr == 0, s_src, s_src - cfg.HALF).astype(np.int16)
    idx_all[c_arr, slot] = idx16
    dst_all[c_arr, slot] = (s_dst % R - w_arr * cfg.WROWS).astype(np.float32)
    nrm_all[c_arr, slot] = s_norm.astype(np.float32)

    # per-(window,group) valid counts; guard empty groups with one idx-0 slot
    import os
    idx_all[idx_all < 0] = 0
    gcnt = np.ones((cfg.n_cores, 1, max(n_win * 2, 16)), np.int32)

    # wrap idx into [128, S/16] (16-part wrap, replicated x8)
    j = np.arange(S_total)
    idx_wrapped = np.zeros((cfg.n_cores, 16, S_total // 16), np.int16)
    idx_wrapped[:, j % 16, j // 16] = idx_all
    idx_wrapped = np.tile(idx_wrapped, (1, 8, 1))  # [cores, 128, S/16]
    # dst/nrm into [128, T_total]
    dst_t = dst_all.reshape(cfg.n_cores, T_total, 128).transpose(0, 2, 1).copy()
    nrm_t = nrm_all.reshape(cfg.n_cores, T_total, 128).transpose(0, 2, 1).copy()

    # dense weights, sbuf layouts
    KH = cfg.IN // 128
    wc_sb = np.ascontiguousarray(
        W_c.reshape(KH, 128, cfg.HID).transpose(1, 0, 2)
    ).astype(np.float16)  # [128, KH, HID]
    KH2 = cfg.HID // 128
    w2_sb = np.ascontiguousarray(
        np.asarray(W2, np.float32).reshape(KH2, 128, cfg.OUT).transpose(1, 0, 2)
    ).astype(np.float16)  # [128, KH2, OUT]
    biasw = np.stack([b_c, b1]).astype(np.float16)  # [2, HID]
    b2b = np.tile(b2[None, :], (128, 1)).astype(np.float32)  # [128, OUT]
    iota = np.tile(np.arange(128, dtype=np.float16)[None, :], (128, 1))
    ident = np.eye(128, dtype=np.float16)
    r_loc = np.stack(
        [r_vec.reshape(cfg.n_cores, R),
         np.ones((cfg.n_cores, R), np.float32)], axis=1
    ).astype(np.float16)  # [cores, 2, R]

    x_f16 = np.asarray(x, np.float32).astype(np.float16)

    shared = dict(x=x_f16, wc=wc_sb, w2=w2_sb, biasw=biasw, b2b=b2b,
                  iota=iota, ident=ident)
    per_core = [
        dict(idx=idx_wrapped[c], dstv=dst_t[c], nrmv=nrm_t[c], rv=r_loc[c],
             gcnt=gcnt[c])
        for c in range(cfg.n_cores)
    ]
    meta = dict(TAs=[int(v) for v in TAs], TBs=[int(v) for v in TBs],
                TWmax=TWmax, S_total=S_total, T_total=T_total)
    return shared, per_core, meta


def build(cfg: Cfg, meta, reps: int = 1, single_core_sim: bool = False) -> bass.Bass:
    TAs, TBs, TWmax = meta["TAs"], meta["TBs"], meta["TWmax"]
    TWs = [a + b for a, b in zip(TAs, TBs)]
    tile_base = [0]
    for tw in TWs:
        tile_base.append(tile_base[-1] + tw)
    S_total, T_total = meta["S_total"], meta["T_total"]
    N, R, IN, HID, OUT = cfg.N, cfg.R, cfg.IN, cfg.HID, cfg.OUT
    n_win = cfg.n_win
    KH, KH2 = IN // 128, HID // 128
    MT, MT2 = HID // 128, OUT // 128
    n_chunk = (R + cfg.CHUNK - 1) // cfg.CHUNK

    nc = bacc.Bacc("TRN2", target_bir_lowering=False, debug=False,
                   num_devices=1 if single_core_sim else cfg.n_cores)

    # ---- I/O ----
    x_d = nc.dram_tensor("x", [N, IN], F16, kind="ExternalInput")
    idx_d = nc.dram_tensor("idx", [128, S_total // 16], I16, kind="ExternalInput")
    dst_d = nc.dram_tensor("dstv", [128, T_total], F32, kind="ExternalInput")
    nrm_d = nc.dram_tensor("nrmv", [128, T_total], F32, kind="ExternalInput")
    rv_d = nc.dram_tensor("rv", [2, R], F16, kind="ExternalInput")
    gcnt_d = nc.dram_tensor("gcnt", [1, max(2 * n_win, 16)], mybir.dt.int32,
                            kind="ExternalInput")
    wc_d = nc.dram_tensor("wc", [128, KH, HID], F16, kind="ExternalInput")
    w2_d = nc.dram_tensor("w2", [128, KH2, OUT], F16, kind="ExternalInput")
    biasw_d = nc.dram_tensor("biasw", [2, HID], F16, kind="ExternalInput")
    b2b_d = nc.dram_tensor("b2b", [128, OUT], F32, kind="ExternalInput")
    iota_d = nc.dram_tensor("iota", [128, 128], F16, kind="ExternalInput")
    ident_d = nc.dram_tensor("ident", [128, 128], F16, kind="ExternalInput")
    y_d = nc.dram_tensor("y", [R, OUT], F32, kind="ExternalOutput")

    hw2_loc = nc.dram_tensor("hw2_loc", [R, OUT], F16)
    aspace = "Shared" if (cfg.n_cores > 4 and not single_core_sim) else "Local"
    hw2_full = nc.dram_tensor("hw2_full", [N, OUT], F16, addr_space=aspace)

    # ---- static SBUF ----
    a = nc.alloc_sbuf_tensor
    idx_s = a("idx_s", [128, S_total // 16], I16)
    dst_s = a("dst_s", [128, T_total], F32)
    nrm_s = a("nrm_s", [128, T_total], F32)
    G = a("G", [128, 2 * TWmax, IN], F16)       # gather double buffer
    fmT = a("fmT", [128, max(KH, MT2), R], F16)  # g1T then hw2T (feature-major)
    h1T = a("h1T", [128, MT, R], F16)
    wc_s = a("wc_s", [128, KH, HID], F16)
    w2_s = a("w2_s", [128, KH2, OUT], F16)
    biasw_s = a("biasw_s", [2, HID], F16)
    misc_s = a("misc_s", [2, R], F16)            # row0 = r, row1 = ones
    b2b_s = a("b2b_s", [128, OUT], F32)
    iota_s = a("iota_s", [128, 128], F16)
    gcnt_s = a("gcnt_s", [1, max(2 * n_win, 16)], mybir.dt.int32)
    ident_s = a("ident_s", [128, 128], F16)

    with tile.TileContext(nc) as tc, ExitStack() as ctx:
        p_S = ctx.enter_context(tc.tile_pool(name="p_S", bufs=4))
        p_st = ctx.enter_context(tc.tile_pool(name="p_st", bufs=3))
        p_gw = ctx.enter_context(tc.tile_pool(name="p_gw", bufs=2, space="PSUM"))
        p_tp = ctx.enter_context(tc.tile_pool(name="p_tp", bufs=2, space="PSUM"))
        p_dn = ctx.enter_context(tc.tile_pool(name="p_dn", bufs=2, space="PSUM"))

        # ---- loads ----
        nc.sync.dma_start(out=idx_s[:, :], in_=idx_d[:, :])
        _ = 0  # reps loop below re-traces the body
        nc.sync.dma_start(out=dst_s[:, :], in_=dst_d[:, :])
        nc.sync.dma_start(out=nrm_s[:, :], in_=nrm_d[:, :])
        nc.sync.dma_start(out=wc_s[:, :, :], in_=wc_d[:, :, :])
        nc.sync.dma_start(out=w2_s[:, :, :], in_=w2_d[:, :, :])
        nc.sync.dma_start(out=biasw_s[:, :], in_=biasw_d[:, :])
        nc.sync.dma_start(out=misc_s[:, :], in_=rv_d[:, :])
        nc.sync.dma_start(out=b2b_s[:, :], in_=b2b_d[:, :])
        nc.sync.dma_start(out=iota_s[:, :], in_=iota_d[:, :])
        nc.sync.dma_start(out=gcnt_s[:, :], in_=gcnt_d[:, :])
        nc.vector.memset(G[:, :, :], 0.0)
        nc.sync.dma_start(out=ident_s[:, :], in_=ident_d[:, :])

        nidx_reg = nc.gpsimd.alloc_register("nidx")

        import os
        sent_layers = os.environ.get("GCN_SENT_LAYERS", "")

        def sparse_layer(table_ap_A, table_ap_B, post_window, layer=1):
            use_reg = str(layer) in sent_layers
            """Run all windows of one sparse layer.  post_window(w, rows_w,
            psum_ap) consumes the accumulated [128, IN/OUT] fp32 window."""
            for w in range(n_win):
                rows_w = min(cfg.WROWS, R - w * cfg.WROWS)
                TA, TB = TAs[w], TBs[w]
                TW = TA + TB
                buf = (w % 2) * TWmax
                s0 = tile_base[w] * 128  # first global slot of window
                if TA > 0:
                    nc.gpsimd.dma_gather(
                        out_ap=G[:, buf:buf + TA, :],
                        in_ap=table_ap_A,
                        idxs_ap=idx_s[:, s0 // 16:(s0 + TA * 128) // 16],
                        num_idxs=TA * 128,
                        num_idxs_reg=TA * 128,
                        elem_size=IN,
                        single_packet=False,
                    )
                sB = s0 + TA * 128
                if TB > 0:
                    nc.gpsimd.dma_gather(
                        out_ap=G[:, buf + TA:buf + TW, :],
                        in_ap=table_ap_B,
                        idxs_ap=idx_s[:, sB // 16:(sB + TB * 128) // 16],
                        num_idxs=TB * 128,
                        num_idxs_reg=TB * 128,
                        elem_size=IN,
                        single_packet=False,
                    )
                gwin = p_gw.tile([128, IN], F32, tag="gwin")
                for t in range(TW):
                    tg = tile_base[w] + t
                    S_t = p_S.tile([128, 128], F16, tag="S")
                    nc.vector.tensor_scalar(
                        out=S_t[:, :], in0=iota_s[:, :],
                        scalar1=dst_s[:, tg:tg + 1],
                        scalar2=nrm_s[:, tg:tg + 1],
                        op0=mybir.AluOpType.is_equal,
                        op1=mybir.AluOpType.mult,
                    )
                    nc.tensor.matmul(
                        out=gwin[:, :], lhsT=S_t[:, :], rhs=G[:, buf + t, :],
                        start=(t == 0), stop=(t == TW - 1),
                    )
                post_window(w, rows_w, gwin)

        # ================= layer 1 =================
        def post1(w, rows_w, gwin):
            st = p_st.tile([128, IN], F16, tag="st")
            nc.vector.tensor_copy(st[:, :], gwin[:, :])
            r0 = w * cfg.WROWS
            for fh in range(KH):
                tp = p_tp.tile([128, 128], F16, tag="tp")
                nc.tensor.transpose(tp[:, :], st[:, fh * 128:(fh + 1) * 128],
                                    ident_s[:, :])
                nc.vector.tensor_copy(fmT[:, fh, r0:r0 + rows_w],
                                      tp[:, :rows_w])

        def body():
            sparse_layer(x_d[:, :], x_d[cfg.HALF:, :], post1, layer=1)

            # dense: h1T = sigmoid(Wc^T g1T + biases)
            for ch in range(n_chunk):
                c0 = ch * cfg.CHUNK
                cw = min(cfg.CHUNK, R - c0)
                for mt in range(MT):
                    ps = p_dn.tile([128, cfg.CHUNK], F32, tag="dn")
                    for kh in range(KH):
                        nc.tensor.matmul(
                            out=ps[:, :cw],
                            lhsT=wc_s[:, kh, mt * 128:(mt + 1) * 128],
                            rhs=fmT[:, kh, c0:c0 + cw],
                            start=(kh == 0), stop=False,
                        )
                    nc.tensor.matmul(
                        out=ps[:, :cw],
                        lhsT=biasw_s[:, mt * 128:(mt + 1) * 128],
                        rhs=misc_s[:, c0:c0 + cw],
                        start=False, stop=True,
                    )
                    nc.scalar.activation(h1T[:, mt, c0:c0 + cw], ps[:, :cw],
                                         mybir.ActivationFunctionType.Sigmoid)

            # dense: hw2T = W2^T h1T   (fmT reused as hw2T)
            for ch in range(n_chunk):
                c0 = ch * cfg.CHUNK
                cw = min(cfg.CHUNK, R - c0)
                for mt in range(MT2):
                    ps = p_dn.tile([128, cfg.CHUNK], F32, tag="dn")
                    for kh in range(KH2):
                        nc.tensor.matmul(
                            out=ps[:, :cw],
                            lhsT=w2_s[:, kh, mt * 128:(mt + 1) * 128],
                            rhs=h1T[:, kh, c0:c0 + cw],
                            start=(kh == 0), stop=(kh == KH2 - 1),
                        )
                    nc.vector.tensor_copy(fmT[:, mt, c0:c0 + cw], ps[:, :cw])

            # transpose hw2T -> node-major (staged in G, idle here), DMA, AllGather
            assert OUT == IN and n_win <= 2 * TWmax
            hw2n = G
            for nt in range(n_win):
                n0 = nt * 128
                cn = min(128, R - n0)
                for fh in range(MT2):
                    tp = p_tp.tile([128, 128], F16, tag="tp")
                    nc.tensor.transpose(tp[:cn, :], fmT[:, fh, n0:n0 + cn],
                                        ident_s[:, :])
                    nc.vector.tensor_copy(hw2n[:cn, nt, fh * 128:(fh + 1) * 128],
                                          tp[:cn, :])
            nfull = (R // 128) * 128
            nc.sync.dma_start(
                out=hw2_loc[0:nfull, :].rearrange("(n p) f -> p n f", p=128),
                in_=hw2n[:, 0:R // 128, :])
            if R % 128:
                nc.sync.dma_start(out=hw2_loc[nfull:R, :],
                                  in_=hw2n[0:R - nfull, R // 128, :])
            if single_core_sim:
                nc.sync.dma_start(out=hw2_full[0:R, :], in_=hw2_loc[:, :])
            else:
                nc.gpsimd.collective_compute(
                    "AllGather", mybir.AluOpType.bypass,
                    replica_groups=[list(range(cfg.n_cores))],
                    ins=[hw2_loc[:, :]], outs=[hw2_full[:, :]],
                )

            # ================= layer 2 =================
            def post2(w, rows_w, gwin):
                st = p_st.tile([128, OUT], F32, tag="st2")
                nc.vector.tensor_tensor(
                    out=st[:rows_w, :], in0=gwin[:rows_w, :],
                    in1=b2b_s[:rows_w, :], op=mybir.AluOpType.add)
                so = p_st.tile([128, OUT], F32, tag="so")
                nc.scalar.activation(so[:rows_w, :], st[:rows_w, :],
                                     mybir.ActivationFunctionType.Sigmoid)
                r0 = w * cfg.WROWS
                nc.sync.dma_start(out=y_d[r0:r0 + rows_w, :], in_=so[:rows_w, :])

            sparse_layer(hw2_full[:, :], hw2_full[cfg.HALF:, :], post2, layer=2)

        for _rep in range(reps):
            body()

    nc.compile()
    return nc


def run(cfg: Cfg, inputs, use_sim=False, trace=False):
    shared, per_core, meta = preprocess(cfg, **inputs)
    nc = build(cfg, meta)
    in_maps = [{**shared, **pc} for pc in per_core]
    if use_sim:
        from concourse.bass_interp import MultiCoreSim
        sim = MultiCoreSim(nc, num_cores=cfg.n_cores)
        for c in range(cfg.n_cores):
            for k, v in in_maps[c].items():
                sim.cores[c].tensor(k)[:] = v
        sim.simulate(check_with_hw=False)
        outs = [np.array(sim.cores[c].tensor("y")) for c in range(cfg.n_cores)]
        return np.concatenate(outs, 0), None
    else:
        from concourse import bass_utils
        res = bass_utils.run_bass_kernel_spmd(
            nc, in_maps, core_ids=list(range(cfg.n_cores)), trace=trace)
        outs = [r["y"] for r in res.results]
        return np.concatenate(outs, 0), res


# ----------------------------------------------------------------------------
# harness entry point
# ----------------------------------------------------------------------------
_CACHE = {}


def _get_compiled(meta):
    key = (tuple(meta["TAs"]), tuple(meta["TBs"]))
    if key not in _CACHE:
        cfg = Cfg()
        _CACHE[key] = build(cfg, meta)
    return _CACHE[key]


def kernel(**inputs) -> np.ndarray:
    """Full GCN encoder on 8 trn2 cores.  Takes full unsharded inputs,
    returns the full [N, OUT] float32 output."""
    cfg = Cfg()
    inputs = {k: np.asarray(v) for k, v in inputs.items()}
    shared, per_core, meta = preprocess(cfg, **inputs)
    nc = _get_compiled(meta)
    in_maps = [{**shared, **pc} for pc in per_core]
    from concourse import bass_utils
    res = bass_utils.run_bass_kernel_spmd(
        nc, in_maps, core_ids=list(range(cfg.n_cores)))
    return np.concatenate([r["y"] for r in res.results], 0)



# revision 3
# speedup vs baseline: 1.9276x; 1.9276x over previous
"""GCN encoder kernel for trn2 (8 cores).

Math (reference):
    src,dst,norm = gcn_norm(edge_index, N)   # self loops + sym norm
    h  = x @ W_ae + b_ae
    h1 = sigmoid(Ahat @ (h @ W1) + b1)
    y  = sigmoid(Ahat @ (h1 @ W2) + b2)

Restructured (identical math):
    W_c = W_ae @ W1, b_c = b_ae @ W1, r = Ahat @ 1
    g1 = Ahat @ x                       # sparse, 256-dim gather
    h1 = sigmoid(g1 @ W_c + r b_c^T + 1 b1^T)
    hw2 = h1 @ W2
    y  = sigmoid(Ahat @ hw2 + 1 b2^T)   # sparse, 256-dim gather

Sharding: core c owns dst rows [c*R, (c+1)*R).  One AllGather of hw2.
Sparse phase: dma_gather rows from HBM, segment-sum via PE matmul with
per-tile selection matrices S (norm folded in) built on DVE.
"""

import dataclasses
import math
from contextlib import ExitStack

import numpy as np

import concourse.bass as bass
import concourse.mybir as mybir
import concourse.tile as tile
from concourse import bacc

F16 = mybir.dt.float16
F32 = mybir.dt.float32
I16 = mybir.dt.int16


@dataclasses.dataclass
class Cfg:
    N: int = 50000
    E: int = 1600000
    IN: int = 256          # gather feature dim (x and hw2)
    HID: int = 512         # h1 dim
    OUT: int = 256         # output dim
    n_cores: int = 8
    HALF: int = 32768      # int16 index split point
    WROWS: int = 128       # dst rows per window
    CHUNK: int = 512       # dense matmul node-chunk (PSUM free dim)

    @property
    def R(self):
        assert self.N % self.n_cores == 0
        return self.N // self.n_cores

    @property
    def n_win(self):
        return (self.R + self.WROWS - 1) // self.WROWS


def preprocess(cfg: Cfg, x, edge_index, W_ae, b_ae, W1, b1, W2, b2):
    """Host-side: graph norm, weight folding, per-core edge layout."""
    N, R = cfg.N, cfg.R
    src = np.asarray(edge_index[0], dtype=np.int64)
    dst = np.asarray(edge_index[1], dtype=np.int64)
    loops = np.arange(N, dtype=np.int64)
    src = np.concatenate([src, loops])
    dst = np.concatenate([dst, loops])
    deg = np.bincount(dst, minlength=N).astype(np.float32)
    dinv = np.where(deg > 0, 1.0 / np.sqrt(deg), 0.0).astype(np.float32)
    norm = (dinv[src] * dinv[dst]).astype(np.float32)
    r_vec = np.bincount(dst, weights=norm.astype(np.float64), minlength=N)
    r_vec = r_vec.astype(np.float32)

    W_c = (np.asarray(W_ae, np.float32) @ np.asarray(W1, np.float32))
    b_c = (np.asarray(b_ae, np.float32) @ np.asarray(W1, np.float32))
    b1 = np.asarray(b1, np.float32)
    b2 = np.asarray(b2, np.float32)

    # per (core, window, group) edge lists
    core_of = dst // R
    wrow = (dst % R) // cfg.WROWS
    grp = (src >= cfg.HALF).astype(np.int64)
    n_win = cfg.n_win
    # count per (core, window, group)
    key = (core_of * n_win + wrow) * 2 + grp
    counts = np.bincount(key, minlength=cfg.n_cores * n_win * 2).reshape(
        cfg.n_cores, n_win, 2
    )
    # per-window tile counts, max over cores (uniform SPMD program)
    TAs = np.ceil(counts[:, :, 0].max(axis=0) / 128).astype(np.int64)  # [n_win]
    TBs = np.ceil(counts[:, :, 1].max(axis=0) / 128).astype(np.int64)
    TWs = TAs + TBs
    tile_base = np.zeros(n_win + 1, np.int64)
    tile_base[1:] = np.cumsum(TWs)
    T_total = int(tile_base[-1])
    S_total = T_total * 128
    # pad S_total to a multiple of 16 cols is automatic (128 | S_total)
    TWmax = int(TWs.max())

    # order edges by key then src (sorted src within group => quasi-seq HBM reads)
    order = np.lexsort((src, key))
    s_src, s_dst, s_norm, s_key = src[order], dst[order], norm[order], key[order]
    # slot id within each key block
    blk_start = np.searchsorted(s_key, np.arange(cfg.n_cores * n_win * 2))
    within = np.arange(len(s_src)) - blk_start[s_key]

    idx_all = np.full((cfg.n_cores, S_total), -1, np.int16)
    dst_all = np.zeros((cfg.n_cores, S_total), np.float32)
    nrm_all = np.zeros((cfg.n_cores, S_total), np.float32)

    c_arr = s_key // (2 * n_win)
    w_arr = (s_key // 2) % n_win
    g_arr = s_key % 2
    slot = (tile_base[w_arr] + g_arr * TAs[w_arr]) * 128 + within
    idx16 = np.where(g_arr == 0, s_src, s_src - cfg.HALF).astype(np.int16)
    idx_all[c_arr, slot] = idx16
    dst_all[c_arr, slot] = (s_dst % R - w_arr * cfg.WROWS).astype(np.float32)
    nrm_all[c_arr, slot] = s_norm.astype(np.float32)

    # per-(window,group) valid counts; guard empty groups with one idx-0 slot
    import os
    idx_all[idx_all < 0] = 0
    gcnt = np.ones((cfg.n_cores, 1, max(n_win * 2, 16)), np.int32)

    # wrap idx into [128, S/16] (16-part wrap, replicated x8)
    j = np.arange(S_total)
    idx_wrapped = np.zeros((cfg.n_cores, 16, S_total // 16), np.int16)
    idx_wrapped[:, j % 16, j // 16] = idx_all
    idx_wrapped = np.tile(idx_wrapped, (1, 8, 1))  # [cores, 128, S/16]
    # dst/nrm into [128, T_total]
    dst_t = dst_all.reshape(cfg.n_cores, T_total, 128).transpose(0, 2, 1).copy()
    nrm_t = nrm_all.reshape(cfg.n_cores, T_total, 128).transpose(0, 2, 1).copy()

    # dense weights, sbuf layouts
    KH = cfg.IN // 128
    wc_sb = np.ascontiguousarray(
        W_c.reshape(KH, 128, cfg.HID).transpose(1, 0, 2)
    ).astype(np.float16)  # [128, KH, HID]
    KH2 = cfg.HID // 128
    w2_sb = np.ascontiguousarray(
        np.asarray(W2, np.float32).reshape(KH2, 128, cfg.OUT).transpose(1, 0, 2)
    ).astype(np.float16)  # [128, KH2, OUT]
    biasw = np.stack([b_c, b1]).astype(np.float16)  # [2, HID]
    b2b = np.tile(b2[None, :], (128, 1)).astype(np.float32)  # [128, OUT]
    iota = np.tile(np.arange(128, dtype=np.float16)[None, :], (128, 1))
    ident = np.eye(128, dtype=np.float16)
    r_loc = np.stack(
        [r_vec.reshape(cfg.n_cores, R),
         np.ones((cfg.n_cores, R), np.float32)], axis=1
    ).astype(np.float16)  # [cores, 2, R]

    x_f16 = np.asarray(x, np.float32).astype(np.float16)

    shared = dict(x=x_f16, wc=wc_sb, w2=w2_sb, biasw=biasw, b2b=b2b,
                  iota=iota, ident=ident)
    per_core = [
        dict(idx=idx_wrapped[c], dstv=dst_t[c], nrmv=nrm_t[c], rv=r_loc[c],
             gcnt=gcnt[c])
        for c in range(cfg.n_cores)
    ]
    meta = dict(TAs=[int(v) for v in TAs], TBs=[int(v) for v in TBs],
                TWmax=TWmax, S_total=S_total, T_total=T_total)
    return shared, per_core, meta


def build(cfg: Cfg, meta, reps: int = 1, single_core_sim: bool = False) -> bass.Bass:
    TAs, TBs, TWmax = meta["TAs"], meta["TBs"], meta["TWmax"]
    TWs = [a + b for a, b in zip(TAs, TBs)]
    tile_base = [0]
    for tw in TWs:
        tile_base.append(tile_base[-1] + tw)
    S_total, T_total = meta["S_total"], meta["T_total"]
    N, R, IN, HID, OUT = cfg.N, cfg.R, cfg.IN, cfg.HID, cfg.OUT
    n_win = cfg.n_win
    KH, KH2 = IN // 128, HID // 128
    MT, MT2 = HID // 128, OUT // 128
    n_chunk = (R + cfg.CHUNK - 1) // cfg.CHUNK

    nc = bacc.Bacc("TRN2", target_bir_lowering=False, debug=False,
                   num_devices=1 if single_core_sim else cfg.n_cores,
                   num_swdge_queues=4)

    # ---- I/O ----
    x_d = nc.dram_tensor("x", [N, IN], F16, kind="ExternalInput")
    idx_d = nc.dram_tensor("idx", [128, S_total // 16], I16, kind="ExternalInput")
    dst_d = nc.dram_tensor("dstv", [128, T_total], F32, kind="ExternalInput")
    nrm_d = nc.dram_tensor("nrmv", [128, T_total], F32, kind="ExternalInput")
    rv_d = nc.dram_tensor("rv", [2, R], F16, kind="ExternalInput")
    gcnt_d = nc.dram_tensor("gcnt", [1, max(2 * n_win, 16)], mybir.dt.int32,
                            kind="ExternalInput")
    wc_d = nc.dram_tensor("wc", [128, KH, HID], F16, kind="ExternalInput")
    w2_d = nc.dram_tensor("w2", [128, KH2, OUT], F16, kind="ExternalInput")
    biasw_d = nc.dram_tensor("biasw", [2, HID], F16, kind="ExternalInput")
    b2b_d = nc.dram_tensor("b2b", [128, OUT], F32, kind="ExternalInput")
    iota_d = nc.dram_tensor("iota", [128, 128], F16, kind="ExternalInput")
    ident_d = nc.dram_tensor("ident", [128, 128], F16, kind="ExternalInput")
    y_d = nc.dram_tensor("y", [R, OUT], F32, kind="ExternalOutput")

    hw2_loc = nc.dram_tensor("hw2_loc", [R, OUT], F16)
    aspace = "Shared" if (cfg.n_cores > 4 and not single_core_sim) else "Local"
    hw2_full = nc.dram_tensor("hw2_full", [N, OUT], F16, addr_space=aspace)

    # ---- static SBUF ----
    a = nc.alloc_sbuf_tensor
    idx_s = a("idx_s", [128, S_total // 16], I16)
    dst_s = a("dst_s", [128, T_total], F32)
    nrm_s = a("nrm_s", [128, T_total], F32)
    G = a("G", [128, 2 * TWmax, IN], F16)       # gather double buffer
    fmT = a("fmT", [128, max(KH, MT2), R], F16)  # g1T then hw2T (feature-major)
    h1T = a("h1T", [128, MT, R], F16)
    wc_s = a("wc_s", [128, KH, HID], F16)
    w2_s = a("w2_s", [128, KH2, OUT], F16)
    biasw_s = a("biasw_s", [2, HID], F16)
    misc_s = a("misc_s", [2, R], F16)            # row0 = r, row1 = ones
    b2b_s = a("b2b_s", [128, OUT], F32)
    iota_s = a("iota_s", [128, 128], F16)
    gcnt_s = a("gcnt_s", [1, max(2 * n_win, 16)], mybir.dt.int32)
    ident_s = a("ident_s", [128, 128], F16)

    with tile.TileContext(nc) as tc, ExitStack() as ctx:
        p_S = ctx.enter_context(tc.tile_pool(name="p_S", bufs=4))
        p_st = ctx.enter_context(tc.tile_pool(name="p_st", bufs=3))
        p_gw = ctx.enter_context(tc.tile_pool(name="p_gw", bufs=2, space="PSUM"))
        p_tp = ctx.enter_context(tc.tile_pool(name="p_tp", bufs=2, space="PSUM"))
        p_dn = ctx.enter_context(tc.tile_pool(name="p_dn", bufs=2, space="PSUM"))

        # ---- loads ----
        nc.sync.dma_start(out=idx_s[:, :], in_=idx_d[:, :])
        _ = 0  # reps loop below re-traces the body
        nc.sync.dma_start(out=dst_s[:, :], in_=dst_d[:, :])
        nc.sync.dma_start(out=nrm_s[:, :], in_=nrm_d[:, :])
        nc.sync.dma_start(out=wc_s[:, :, :], in_=wc_d[:, :, :])
        nc.sync.dma_start(out=w2_s[:, :, :], in_=w2_d[:, :, :])
        nc.sync.dma_start(out=biasw_s[:, :], in_=biasw_d[:, :])
        nc.sync.dma_start(out=misc_s[:, :], in_=rv_d[:, :])
        nc.sync.dma_start(out=b2b_s[:, :], in_=b2b_d[:, :])
        nc.sync.dma_start(out=iota_s[:, :], in_=iota_d[:, :])
        nc.sync.dma_start(out=gcnt_s[:, :], in_=gcnt_d[:, :])
        nc.vector.memset(G[:, :, :], 0.0)
        nc.sync.dma_start(out=ident_s[:, :], in_=ident_d[:, :])

        nidx_reg = nc.gpsimd.alloc_register("nidx")

        import os
        sent_layers = os.environ.get("GCN_SENT_LAYERS", "")

        def sparse_layer(table_ap_A, table_ap_B, post_window, layer=1):
            use_reg = str(layer) in sent_layers
            """Run all windows of one sparse layer.  post_window(w, rows_w,
            psum_ap) consumes the accumulated [128, IN/OUT] fp32 window."""
            for w in range(n_win):
                rows_w = min(cfg.WROWS, R - w * cfg.WROWS)
                TA, TB = TAs[w], TBs[w]
                TW = TA + TB
                buf = (w % 2) * TWmax
                s0 = tile_base[w] * 128  # first global slot of window
                if TA > 0:
                    nc.gpsimd.dma_gather(
                        out_ap=G[:, buf:buf + TA, :],
                        in_ap=table_ap_A,
                        idxs_ap=idx_s[:, s0 // 16:(s0 + TA * 128) // 16],
                        num_idxs=TA * 128,
                        num_idxs_reg=TA * 128,
                        elem_size=IN,
                        single_packet=False,
                        queue_num=(2 * w) % 4,
                    )
                sB = s0 + TA * 128
                if TB > 0:
                    nc.gpsimd.dma_gather(
                        out_ap=G[:, buf + TA:buf + TW, :],
                        in_ap=table_ap_B,
                        idxs_ap=idx_s[:, sB // 16:(sB + TB * 128) // 16],
                        num_idxs=TB * 128,
                        num_idxs_reg=TB * 128,
                        elem_size=IN,
                        single_packet=False,
                        queue_num=(2 * w + 1) % 4,
                    )
                gwin = p_gw.tile([128, IN], F32, tag="gwin")
                for t in range(TW):
                    tg = tile_base[w] + t
                    S_t = p_S.tile([128, 128], F16, tag="S")
                    nc.vector.tensor_scalar(
                        out=S_t[:, :], in0=iota_s[:, :],
                        scalar1=dst_s[:, tg:tg + 1],
                        scalar2=nrm_s[:, tg:tg + 1],
                        op0=mybir.AluOpType.is_equal,
                        op1=mybir.AluOpType.mult,
                    )
                    nc.tensor.matmul(
                        out=gwin[:, :], lhsT=S_t[:, :], rhs=G[:, buf + t, :],
                        start=(t == 0), stop=(t == TW - 1),
                    )
                post_window(w, rows_w, gwin)

        # ================= layer 1 =================
        def post1(w, rows_w, gwin):
            st = p_st.tile([128, IN], F16, tag="st")
            nc.vector.tensor_copy(st[:, :], gwin[:, :])
            r0 = w * cfg.WROWS
            for fh in range(KH):
                tp = p_tp.tile([128, 128], F16, tag="tp")
                nc.tensor.transpose(tp[:, :], st[:, fh * 128:(fh + 1) * 128],
                                    ident_s[:, :])
                nc.vector.tensor_copy(fmT[:, fh, r0:r0 + rows_w],
                                      tp[:, :rows_w])

        def body():
            sparse_layer(x_d[:, :], x_d[cfg.HALF:, :], post1, layer=1)

            # dense: h1T = sigmoid(Wc^T g1T + biases)
            for ch in range(n_chunk):
                c0 = ch * cfg.CHUNK
                cw = min(cfg.CHUNK, R - c0)
                for mt in range(MT):
                    ps = p_dn.tile([128, cfg.CHUNK], F32, tag="dn")
                    for kh in range(KH):
                        nc.tensor.matmul(
                            out=ps[:, :cw],
                            lhsT=wc_s[:, kh, mt * 128:(mt + 1) * 128],
                            rhs=fmT[:, kh, c0:c0 + cw],
                            start=(kh == 0), stop=False,
                        )
                    nc.tensor.matmul(
                        out=ps[:, :cw],
                        lhsT=biasw_s[:, mt * 128:(mt + 1) * 128],
                        rhs=misc_s[:, c0:c0 + cw],
                        start=False, stop=True,
                    )
                    nc.scalar.activation(h1T[:, mt, c0:c0 + cw], ps[:, :cw],
                                         mybir.ActivationFunctionType.Sigmoid)

            # dense: hw2T = W2^T h1T   (fmT reused as hw2T)
            for ch in range(n_chunk):
                c0 = ch * cfg.CHUNK
                cw = min(cfg.CHUNK, R - c0)
                for mt in range(MT2):
                    ps = p_dn.tile([128, cfg.CHUNK], F32, tag="dn")
                    for kh in range(KH2):
                        nc.tensor.matmul(
                            out=ps[:, :cw],
                            lhsT=w2_s[:, kh, mt * 128:(mt + 1) * 128],
                            rhs=h1T[:, kh, c0:c0 + cw],
                            start=(kh == 0), stop=(kh == KH2 - 1),
                        )
                    nc.vector.tensor_copy(fmT[:, mt, c0:c0 + cw], ps[:, :cw])

            # transpose hw2T -> node-major (staged in G, idle here), DMA, AllGather
            assert OUT == IN and n_win <= 2 * TWmax
            hw2n = G
            for nt in range(n_win):
                n0 = nt * 128
                cn = min(128, R - n0)
                for fh in range(MT2):
                    tp = p_tp.tile([128, 128], F16, tag="tp")
                    nc.tensor.transpose(tp[:cn, :], fmT[:, fh, n0:n0 + cn],
                                        ident_s[:, :])
                    nc.vector.tensor_copy(hw2n[:cn, nt, fh * 128:(fh + 1) * 128],
                                          tp[:cn, :])
            nfull = (R // 128) * 128
            nc.sync.dma_start(
                out=hw2_loc[0:nfull, :].rearrange("(n p) f -> p n f", p=128),
                in_=hw2n[:, 0:R // 128, :])
            if R % 128:
                nc.sync.dma_start(out=hw2_loc[nfull:R, :],
                                  in_=hw2n[0:R - nfull, R // 128, :])
            if single_core_sim:
                nc.sync.dma_start(out=hw2_full[0:R, :], in_=hw2_loc[:, :])
            else:
                nc.gpsimd.collective_compute(
                    "AllGather", mybir.AluOpType.bypass,
                    replica_groups=[list(range(cfg.n_cores))],
                    ins=[hw2_loc[:, :]], outs=[hw2_full[:, :]],
                )

            # ================= layer 2 =================
            def post2(w, rows_w, gwin):
                st = p_st.tile([128, OUT], F32, tag="st2")
                nc.vector.tensor_tensor(
                    out=st[:rows_w, :], in0=gwin[:rows_w, :],
                    in1=b2b_s[:rows_w, :], op=mybir.AluOpType.add)
                so = p_st.tile([128, OUT], F32, tag="so")
                nc.scalar.activation(so[:rows_w, :], st[:rows_w, :],
                                     mybir.ActivationFunctionType.Sigmoid)
                r0 = w * cfg.WROWS
                nc.sync.dma_start(out=y_d[r0:r0 + rows_w, :], in_=so[:rows_w, :])

            sparse_layer(hw2_full[:, :], hw2_full[cfg.HALF:, :], post2, layer=2)

        for _rep in range(reps):
            body()

    nc.compile()
    return nc


def run(cfg: Cfg, inputs, use_sim=False, trace=False):
    shared, per_core, meta = preprocess(cfg, **inputs)
    nc = build(cfg, meta)
    in_maps = [{**shared, **pc} for pc in per_core]
    if use_sim:
        from concourse.bass_interp import MultiCoreSim
        sim = MultiCoreSim(nc, num_cores=cfg.n_cores)
        for c in range(cfg.n_cores):
            for k, v in in_maps[c].items():
                sim.cores[c].tensor(k)[:] = v
        sim.simulate(check_with_hw=False)
        outs = [np.array(sim.cores[c].tensor("y")) for c in range(cfg.n_cores)]
        return np.concatenate(outs, 0), None
    else:
        from concourse import bass_utils
        res = bass_utils.run_bass_kernel_spmd(
            nc, in_maps, core_ids=list(range(cfg.n_cores)), trace=trace)
        outs = [r["y"] for r in res.results]
        return np.concatenate(outs, 0), res


# ----------------------------------------------------------------------------
# harness entry point
# ----------------------------------------------------------------------------
_CACHE = {}


def _get_compiled(meta):
    key = (tuple(meta["TAs"]), tuple(meta["TBs"]))
    if key not in _CACHE:
        cfg = Cfg()
        _CACHE[key] = build(cfg, meta)
    return _CACHE[key]


def kernel(**inputs) -> np.ndarray:
    """Full GCN encoder on 8 trn2 cores.  Takes full unsharded inputs,
    returns the full [N, OUT] float32 output."""
    cfg = Cfg()
    inputs = {k: np.asarray(v) for k, v in inputs.items()}
    shared, per_core, meta = preprocess(cfg, **inputs)
    nc = _get_compiled(meta)
    in_maps = [{**shared, **pc} for pc in per_core]
    from concourse import bass_utils
    res = bass_utils.run_bass_kernel_spmd(
        nc, in_maps, core_ids=list(range(cfg.n_cores)))
    return np.concatenate([r["y"] for r in res.results], 0)

